# revision 1
# baseline (speedup 1.0000x reference)
"""Trainium2 Bass kernel for nn_Mlp_13099650253522 (BitNet-ternary dense MLP).

  h = gelu(x @ ter_quant(w1).T + b1);  y = h @ ter_quant(w2).T + b2
  ter_quant(w) = clip(round(w / g), -1, 1) * g,  g = mean(|w|) + 1e-5

Strategy (8 NeuronCores, data-parallel over the 64*197=12608 tokens):
 - Host: transpose x/w1/w2 (layout only), shard tokens 1576/core.
 - Device (per core, identical program):
     * gamma: DVE tensor_reduce(|w|) row sums -> GPSIMD partition_all_reduce
       (library pre-warmed with a dummy op at kernel start)
     * ternary quant, exact in fp8 {-2,0,+2}; g/2 folded into epilogues:
       ACT path: Sign(w+g/2)+Sign(w-g/2); DVE path: (w>=g/2)*2-(w<=-g/2)*2,
       split across both engines to shorten the critical path
     * fc1: PE matmul fp8 lhsT x bf16 rhs -> PSUM fp32;
       ACT Gelu(psum * g1/2 + b1) -> bf16
     * fc2: PE matmuls; DVE epilogue psum * g2/2 + b2 -> fp32; DMA out
 - DMA streams serialized w1 -> x -> w2(pass1) -> w2(pass2) via explicit
   deps; w2 moves in 6 big batches per pass for full HBM bandwidth.
 - Host: gather per-core y^T shards, transpose back.
"""
import sys

for _p in ("/root/.axon_site", "/root/.axon_site/_ro/trn_rl_repo",
           "/root/.axon_site/_ro/pypackages", "/opt/trn_rl_repo"):
    if _p not in sys.path:
        sys.path.append(_p)

import ml_dtypes
import numpy as np

from concourse import bacc
import concourse.mybir as mybir
from concourse import bass_isa
from concourse.tile import TileContext
from concourse.tile_rust import add_dep_helper
from concourse.bass_utils import run_bass_kernel_spmd

FP32 = mybir.dt.float32
BF16 = mybir.dt.bfloat16
FP8 = mybir.dt.float8e4
Act = mybir.ActivationFunctionType
Alu = mybir.AluOpType
AxX = mybir.AxisListType.X

N_CORES = 8
B, S, D, H = 64, 197, 768, 3072
TOK = B * S                 # 12608
TOK_PER = TOK // N_CORES    # 1576
NT = 4                      # token tiles per core
TN = TOK_PER // NT          # 394
KD = D // 128               # 6
KH = H // 128               # 24
EPS = 1e-5

W1C = 12                    # w1 load chunks: [128, 1536], 2 per kd
W1_DVE_CHUNKS = 4           # first chunks quantized on DVE (emitted first)
W2B = 6                     # w2 batches: [128, 3072], 4 kh per batch
W2_ACT_BATCHES = 2          # of those, how many quantized on ACT


def build():
    nc = bacc.Bacc("TRN2", target_bir_lowering=False, debug=False)
    xt = nc.declare_dram_parameter("xt", [D, TOK_PER], BF16, isOutput=False)
    wt1 = nc.declare_dram_parameter("wt1", [D, H], FP32, isOutput=False)
    wt2 = nc.declare_dram_parameter("wt2", [H, D], FP32, isOutput=False)
    b1r = nc.declare_dram_parameter("b1r", [128, KH], FP32, isOutput=False)
    b2r = nc.declare_dram_parameter("b2r", [128, KD], FP32, isOutput=False)
    yt = nc.declare_dram_parameter("yt", [D, TOK_PER], FP32, isOutput=True)

    with TileContext(nc) as tc:
        with (
            tc.tile_pool(name="singles", bufs=1) as singles,
            tc.tile_pool(name="wres", bufs=W1C) as wres,    # w1 fp32 resident
            tc.tile_pool(name="wf2", bufs=3) as wf2p,       # w2 fp32 stream
            tc.tile_pool(name="sgnA", bufs=4) as sgnAp,
            tc.tile_pool(name="sgnD", bufs=2) as sgnDp,       # quant transients
            tc.tile_pool(name="t1", bufs=W1C) as t1p,       # ternary w1 fp8
            tc.tile_pool(name="xb", bufs=KD) as xbp,        # x bf16 resident
            tc.tile_pool(name="hb", bufs=44) as hbp,
            tc.tile_pool(name="ysb", bufs=4) as ysbp,
            tc.tile_pool(name="ps", bufs=6, space="PSUM") as psp,
            tc.tile_pool(name="ps2", bufs=2, space="PSUM") as ps2p,
        ):
            # warm the gpsimd custom-op library while w1 streams in
            dmy = singles.tile([128, 1], FP32, tag="dmy")
            nc.gpsimd.memset(dmy, 0.0)
            dmy2 = singles.tile([128, 1], FP32, tag="dmy2")
            nc.gpsimd.partition_all_reduce(dmy2, dmy, channels=128,
                                           reduce_op=bass_isa.ReduceOp.add)

            def gamma_chain(acc_cols, n_cols, total_elems, tag):
                """per-tile |row| sums [128,n] -> (g/2, -g/2) bcast [128,1]"""
                rsum = singles.tile([128, 1], FP32, tag=tag + "_rs")
                nc.vector.tensor_reduce(out=rsum[:, 0:1], in_=acc_cols,
                                        axis=AxX, op=Alu.add)
                allr = singles.tile([128, 1], FP32, tag=tag + "_ar")
                nc.gpsimd.partition_all_reduce(allr, rsum, channels=128,
                                               reduce_op=bass_isa.ReduceOp.add)
                gf = singles.tile([128, 1], FP32, tag=tag + "_gf")
                nc.vector.tensor_scalar(
                    out=gf, in0=allr, scalar1=1.0 / total_elems,
                    scalar2=EPS, op0=Alu.mult, op1=Alu.add)
                gh = singles.tile([128, 1], FP32, tag=tag + "_gh")
                nc.vector.tensor_scalar_mul(gh, gf, 0.5)
                gn = singles.tile([128, 1], FP32, tag=tag + "_gn")
                nc.vector.tensor_scalar_mul(gn, gf, -0.5)
                return gh, gn

            # ---- w1 gamma pass: 12 chunks [128,1536], resident ----
            HC2 = H // 2
            w1t = []
            w1_dmas = []
            acc1 = singles.tile([128, W1C], FP32, tag="acc1")
            for c in range(W1C):
                kd, half = c // 2, c % 2
                wf = wres.tile([128, HC2], FP32, tag="w1")
                w1_dmas.append(nc.sync.dma_start(
                    out=wf, in_=wt1[kd * 128:(kd + 1) * 128,
                                    half * HC2:(half + 1) * HC2]))
                w1t.append(wf)
                nc.vector.tensor_reduce(out=acc1[:, c:c + 1], in_=wf,
                                        axis=AxX, op=Alu.add,
                                        apply_absolute_value=True)
            g1h, g1n = gamma_chain(acc1, W1C, D * H, "g1")

            b1sb = singles.tile([128, KH], FP32, tag="b1sb")
            nc.sync.dma_start(out=b1sb, in_=b1r[:, :])
            b2sb = singles.tile([128, KD], FP32, tag="b2sb")
            nc.sync.dma_start(out=b2sb, in_=b2r[:, :])

            # ---- x load (bf16 from host; gpsimd queue, gated after w1) ----
            xb = []
            xb_dmas = []
            for kd in range(KD):
                xbt = xbp.tile([128, TOK_PER], BF16, tag="xb")
                dma = nc.gpsimd.dma_start(out=xbt,
                                          in_=xt[kd * 128:(kd + 1) * 128, :])
                add_dep_helper(dma.ins, w1_dmas[-1].ins,
                               reason="dma order: x after w1")
                xb_dmas.append(dma)
                xb.append(xbt)

            def xb_slice(kd, t):
                return xb[kd][:, t * TN:(t + 1) * TN]

            def quant_act(wf, t, gh, gn):
                a = sgnAp.tile(list(wf.shape), FP8, tag="sgnA")
                b = sgnAp.tile(list(wf.shape), FP8, tag="sgnA")
                nc.scalar.activation(a, wf, Act.Sign, bias=gh[:, 0:1])
                nc.scalar.activation(b, wf, Act.Sign, bias=gn[:, 0:1])
                nc.vector.tensor_add(out=t, in0=a, in1=b)

            def quant_dve(wf, t, gh, gn):
                a = sgnDp.tile(list(wf.shape), FP8, tag="sgnD")
                b = sgnDp.tile(list(wf.shape), FP8, tag="sgnD")
                nc.vector.tensor_scalar(out=a, in0=wf, scalar1=gh[:, 0:1],
                                        scalar2=2.0, op0=Alu.is_ge,
                                        op1=Alu.mult)
                nc.vector.tensor_scalar(out=b, in0=wf, scalar1=gn[:, 0:1],
                                        scalar2=2.0, op0=Alu.is_le,
                                        op1=Alu.mult)
                nc.vector.tensor_sub(out=t, in0=a, in1=b)

            # ---- w1 quant -> T1 {-2,0,2} fp8, scale g1/2 ----
            t1 = [None] * W1C
            for c in (0, 2, 1, 3, 4, 6, 8, 10, 5, 7, 9, 11):
                t = t1p.tile([128, HC2], FP8, tag="t1")
                if c < W1_DVE_CHUNKS:
                    quant_dve(w1t[c], t, g1h, g1n)
                else:
                    quant_act(w1t[c], t, g1h, g1n)
                t1[c] = t

            def t1_slice(hc, kd):
                # lhsT [128,128] for fc1 group hc, contraction chunk kd
                c = kd * 2 + (hc * 128) // HC2
                off = (hc * 128) % HC2
                return t1[c][:, off:off + 128]

            hbt = {}

            def fc1(t, hcs=range(KH)):
                tok = slice(t * TN, (t + 1) * TN)
                for hc in hcs:
                    ps = psp.tile([128, TN], FP32, tag="hps")
                    order = [(hc + j) % KD for j in range(KD)]
                    for j, kd in enumerate(order):
                        nc.tensor.matmul(ps, t1_slice(hc, kd), xb_slice(kd, t),
                                         start=(j == 0), stop=(j == KD - 1))
                    hbv = hbp.tile([128, TN], BF16, tag="hb")
                    nc.scalar.activation(hbv, ps, Act.Gelu,
                                         bias=b1sb[:, hc:hc + 1],
                                         scale=g1h[:, 0:1])
                    hbt.setdefault(t, []).append(hbv)

            def fc2(t, t2s, g2h):
                tok = slice(t * TN, (t + 1) * TN)
                for dc in range(KD):
                    ps2 = ps2p.tile([128, TN], FP32, tag="yps")
                    order = [(12 + dc + j) % KH for j in range(KH)]
                    for j, kh in enumerate(order):
                        lhsT = t2s[kh // 4][:, kh % 4,
                                            dc * 128:(dc + 1) * 128]
                        nc.tensor.matmul(ps2, lhsT, hbt[t][kh],
                                         start=(j == 0), stop=(j == KH - 1))
                    ysb = ysbp.tile([128, TN], FP32, tag="ysb")
                    nc.vector.tensor_scalar(
                        out=ysb, in0=ps2, scalar1=g2h[:, 0:1],
                        scalar2=b2sb[:, dc:dc + 1],
                        op0=Alu.mult, op1=Alu.add)
                    nc.sync.dma_start(out=yt[dc * 128:(dc + 1) * 128, tok],
                                      in_=ysb)
                del hbt[t]

            # ---- fc1(t0) chases the w1 quant ----
            fc1(0)

            # ---- w2 pass 1: 6 batches [128,3072] (4 kh each), gated ----
            acc2 = singles.tile([128, KH], FP32, tag="acc2")
            w2p1_dmas = []
            w2p1_tiles = []
            for bt in range(W2B):
                wf = wf2p.tile([128, 4, D], FP32, tag="w2")
                src = wt2[bt * 512:(bt + 1) * 512, :]
                dma = nc.sync.dma_start(
                    out=wf, in_=src.rearrange("(c p) f -> p c f", p=128))
                add_dep_helper(dma.ins, xb_dmas[-1].ins,
                               reason="dma order: w2p1 after x")
                w2p1_dmas.append(dma)
                w2p1_tiles.append(wf)
                for c in range(4):
                    nc.vector.tensor_reduce(
                        out=acc2[:, 4 * bt + c:4 * bt + c + 1],
                        in_=wf[:, c, :], axis=AxX, op=Alu.add,
                        apply_absolute_value=True)
            g2h, g2n = gamma_chain(acc2, KH, D * H, "g2")

            # ---- w2 pass 2 (re-read) + quant -> {-2,0,2}, scale g2/2 ----
            t2 = [None] * W2B

            def w2_quant_batch(bt, on_act):
                if bt >= W2B - 3:
                    wf = w2p1_tiles[bt]        # still resident in the ring
                else:
                    wf = wf2p.tile([128, 4, D], FP32, tag="w2")
                    src = wt2[bt * 512:(bt + 1) * 512, :]
                    dma = nc.sync.dma_start(
                        out=wf, in_=src.rearrange("(c p) f -> p c f", p=128))
                    add_dep_helper(dma.ins, w2p1_dmas[-1].ins,
                                   reason="dma order: w2p2 after w2p1")
                t = wres.tile([128, 4, D], FP8, tag="w1")
                if on_act:
                    quant_act(wf, t, g2h, g2n)
                else:
                    quant_dve(wf, t, g2h, g2n)
                t2[bt] = t

            # resident batches first (ready at gamma2), re-reads after;
            # ACT-path ones interleaved between fc1(1) GELU emission
            fc1(1, range(0, 8))
            w2_quant_batch(4, False)
            w2_quant_batch(5, False)
            fc1(1, range(8, 16))
            w2_quant_batch(3, True)
            fc1(1, range(16, 24))
            w2_quant_batch(0, True)
            w2_quant_batch(1, False)
            w2_quant_batch(2, False)

            fc2(0, t2, g2h)
            fc1(2)
            fc2(1, t2, g2h)
            fc1(3)
            fc2(2, t2, g2h)
            fc2(3, t2, g2h)

    nc.compile()
    return nc


_NC = None


def _get_nc():
    global _NC
    if _NC is None:
        _NC = build()
    return _NC


def kernel(x, w1, b1, w2, b2, _trace=False, _trace_kwargs=None):
    nc = _get_nc()
    x = np.asarray(x, dtype=np.float32)
    w1 = np.asarray(w1, dtype=np.float32)
    b1 = np.asarray(b1, dtype=np.float32)
    w2 = np.asarray(w2, dtype=np.float32)
    b2 = np.asarray(b2, dtype=np.float32)
    x2 = np.ascontiguousarray(x.reshape(TOK, D).T).astype(ml_dtypes.bfloat16)
    wt1 = np.ascontiguousarray(w1.T)                    # [768, 3072]
    wt2 = np.ascontiguousarray(w2.T)                    # [3072, 768]
    b1r = np.ascontiguousarray(b1.reshape(KH, 128).T)   # [128, 24]
    b2r = np.ascontiguousarray(b2.reshape(KD, 128).T)   # [128, 6]
    in_maps = []
    for c in range(N_CORES):
        in_maps.append({
            "xt": np.ascontiguousarray(x2[:, c * TOK_PER:(c + 1) * TOK_PER]),
            "wt1": wt1, "wt2": wt2, "b1r": b1r, "b2r": b2r,
        })
    out = run_bass_kernel_spmd(nc, in_maps, list(range(N_CORES)),
                               trace=_trace, **(_trace_kwargs or {}))
    res = out.results
    yt = np.concatenate([res[c]["yt"] for c in range(N_CORES)], axis=1)
    y = np.ascontiguousarray(yt.T).reshape(B, S, D).astype(np.float32)
    if _trace:
        return y, out
    return y



# revision 4
# speedup vs baseline: 1.0399x; 1.0399x over previous
"""Trainium2 Bass kernel for nn_Mlp_13099650253522 (BitNet-ternary dense MLP).

  h = gelu(x @ ter_quant(w1).T + b1);  y = h @ ter_quant(w2).T + b2
  ter_quant(w) = clip(round(w / g), -1, 1) * g,  g = mean(|w|) + 1e-5

Strategy (8 NeuronCores, data-parallel over the 64*197=12608 tokens):
 - Host: transpose + downcast weights to fp16 (layout/dtype only; ternary
   threshold classification verified numerically: rel err ~1.1% < 2e-2),
   x to bf16, shard tokens 1576/core. y returned bf16, upcast on host.
 - Device (per core, identical program):
     * w1 streams in 12 fp16 chunks on the sync DMA queue; per-chunk
       |row| sums on DVE pipelined behind the DMA; fused gamma chain
       (gpsimd partition_all_reduce -> one tensor_scalar for g/2).
     * ternary quant to fp8 {-2,0,+2}; g/2 folded into the matmul
       epilogues. Mostly DVE (fp16 compares run in the DVE 2x mode);
       two chunks on ACT (Sign pairs) to shorten the even-chunk phase.
     * fc1 phase A is chunk-major: 8 PSUM accumulation groups stay open
       and every freshly quantized chunk immediately contributes its
       matmuls, so the PE starts ~2us after gamma instead of waiting
       for six chunks.
     * fc2: PE matmuls fp8 lhsT x bf16 h; DVE epilogue -> bf16 y out.
     * w2 loads once (fp16, host pre-swizzled [128, 24, 768]), reduced
       and quantized in fc1's slack window; no second pass.
 - PE floor is ~189us (bf16 moving operand); everything else is
   scheduled to keep the PE gapless.
"""
import sys

for _p in ("/root/.axon_site", "/root/.axon_site/_ro/trn_rl_repo",
           "/root/.axon_site/_ro/pypackages", "/opt/trn_rl_repo"):
    if _p not in sys.path:
        sys.path.append(_p)

import ml_dtypes
import numpy as np

from concourse import bacc
import concourse.mybir as mybir
from concourse import bass_isa
from concourse.tile import TileContext

FP32 = mybir.dt.float32
FP16 = mybir.dt.float16
BF16 = mybir.dt.bfloat16
FP8 = mybir.dt.float8e4
Act = mybir.ActivationFunctionType
Alu = mybir.AluOpType
AxX = mybir.AxisListType.X

N_CORES = 8
B, S, D, H = 64, 197, 768, 3072
TOK = B * S                 # 12608
TOK_PER = TOK // N_CORES    # 1576
NT = 4                      # token tiles per core
TN = TOK_PER // NT          # 394
KD = D // 128               # 6
KH = H // 128               # 24
EPS = 1e-5

W1C = 12                    # w1 chunks [128, 1536]
HC2 = H // 2
W2B = 6                     # w2 batches [128, 4, 768]
ACT_EVENS = (2, 6)          # even w1 chunks quantized via ACT sign pairs


def build():
    nc = bacc.Bacc("TRN2", target_bir_lowering=False, debug=False)
    xt = nc.declare_dram_parameter("xt", [D, TOK_PER], BF16, isOutput=False)
    wt1 = nc.declare_dram_parameter("wt1", [D, H], FP16, isOutput=False)
    wt2r = nc.declare_dram_parameter("wt2r", [128, KH, D], FP16, isOutput=False)
    b1r = nc.declare_dram_parameter("b1r", [128, KH], FP32, isOutput=False)
    b2r = nc.declare_dram_parameter("b2r", [128, KD], FP32, isOutput=False)
    yt = nc.declare_dram_parameter("yt", [D, TOK_PER], BF16, isOutput=True)

    with TileContext(nc) as tc:
        with (
            tc.tile_pool(name="singles", bufs=1) as singles,
            tc.tile_pool(name="w1p", bufs=W1C) as w1p,       # fp16 w1 resident
            tc.tile_pool(name="t1p", bufs=W1C) as t1p,       # fp8 ternary w1
            tc.tile_pool(name="w2p", bufs=W2B) as w2p,       # fp16 w2 resident
            tc.tile_pool(name="t2p", bufs=W2B) as t2p,       # fp8 ternary w2
            tc.tile_pool(name="xb", bufs=KD) as xbp,         # x bf16 resident
            tc.tile_pool(name="hb", bufs=74) as hbp,         # gelu outputs
            tc.tile_pool(name="scrD", bufs=2) as scrD,       # fp16 cmp scratch
            tc.tile_pool(name="scrA", bufs=2) as scrA,       # fp8 sign scratch
            tc.tile_pool(name="ysb", bufs=3) as ysbp,
            tc.tile_pool(name="ps", bufs=8, space="PSUM") as psp,
        ):
            # warm the gpsimd custom-op library while w1 streams in
            dmy = singles.tile([128, 1], FP32, tag="dmy")
            nc.gpsimd.memset(dmy, 0.0)
            dmy2 = singles.tile([128, 1], FP32, tag="dmy2")
            nc.gpsimd.partition_all_reduce(dmy2, dmy, channels=128,
                                           reduce_op=bass_isa.ReduceOp.add)

            # biases via the gpsimd DMA queue (idle at start)
            b1sb = singles.tile([128, KH], FP32, tag="b1sb")
            nc.gpsimd.dma_start(out=b1sb, in_=b1r[:, :])
            b2sb = singles.tile([128, KD], FP32, tag="b2sb")
            nc.gpsimd.dma_start(out=b2sb, in_=b2r[:, :])

            # ---- w1 DMA (12 fp16 chunks) + pipelined |row| sums ----
            w1t = []
            acc1 = singles.tile([128, W1C], FP32, tag="acc1")
            for c in range(W1C):
                kd, half = c // 2, c % 2
                wf = w1p.tile([128, HC2], FP16, tag="w1")
                nc.sync.dma_start(
                    out=wf, in_=wt1[kd * 128:(kd + 1) * 128,
                                    half * HC2:(half + 1) * HC2])
                w1t.append(wf)
                nc.vector.tensor_reduce(out=acc1[:, c:c + 1], in_=wf,
                                        axis=AxX, op=Alu.add,
                                        apply_absolute_value=True)

            # ---- x DMA (6 bf16 chunks, same queue => after w1) ----
            xb = []
            for kd in range(KD):
                xbt = xbp.tile([128, TOK_PER], BF16, tag="xb")
                nc.sync.dma_start(out=xbt, in_=xt[kd * 128:(kd + 1) * 128, :])
                xb.append(xbt)

            # ---- w2 DMA (6 fp16 batches, after x) ----
            w2t = []
            for bt in range(W2B):
                wf = w2p.tile([128, 4, D], FP16, tag="w2")
                nc.sync.dma_start(out=wf, in_=wt2r[:, 4 * bt:4 * bt + 4, :])
                w2t.append(wf)

            def gamma_half(acc_cols, total_elems, tag):
                """|w| partial sums -> (+g/2, -g/2) broadcast [128,1] fp32."""
                rsum = singles.tile([128, 1], FP32, tag=tag + "_rs")
                nc.vector.tensor_reduce(out=rsum[:, 0:1], in_=acc_cols,
                                        axis=AxX, op=Alu.add)
                allr = singles.tile([128, 1], FP32, tag=tag + "_ar")
                nc.gpsimd.partition_all_reduce(allr, rsum, channels=128,
                                               reduce_op=bass_isa.ReduceOp.add)
                gh = singles.tile([128, 1], FP32, tag=tag + "_gh")
                nc.vector.tensor_scalar(
                    out=gh, in0=allr, scalar1=0.5 / total_elems,
                    scalar2=EPS / 2, op0=Alu.mult, op1=Alu.add)
                gn = singles.tile([128, 1], FP32, tag=tag + "_gn")
                nc.vector.tensor_scalar(
                    out=gn, in0=allr, scalar1=-0.5 / total_elems,
                    scalar2=-EPS / 2, op0=Alu.mult, op1=Alu.add)
                return gh, gn

            # ---- gamma1 ----
            g1h, g1n = gamma_half(acc1, D * H, "g1")

            def quant_dve(wf, t, gh, gn, n):
                """t = (w>=g/2)*2 - (w<=-g/2)*2 via fp16 scratch (DVE 2x)."""
                a = scrD.tile([128, n], FP16, tag="scrD")
                b = scrD.tile([128, n], FP16, tag="scrD")
                nc.vector.tensor_scalar(out=a, in0=wf, scalar1=gh[:, 0:1],
                                        scalar2=2.0, op0=Alu.is_ge,
                                        op1=Alu.mult)
                nc.vector.tensor_scalar(out=b, in0=wf, scalar1=gn[:, 0:1],
                                        scalar2=2.0, op0=Alu.is_le,
                                        op1=Alu.mult)
                nc.vector.tensor_sub(out=t, in0=a, in1=b)

            def quant_act_signs(wf, gh, n):
                """ACT half of quant: a=Sign(w+g/2), b=Sign(-w+g/2)."""
                a = scrA.tile([128, n], FP8, tag="scrA")
                b = scrA.tile([128, n], FP8, tag="scrA")
                nc.scalar.activation(a, wf, Act.Sign, bias=gh[:, 0:1],
                                     scale=1.0)
                nc.scalar.activation(b, wf, Act.Sign, bias=gh[:, 0:1],
                                     scale=-1.0)
                return a, b

            # ---- w1 quant: evens (kd half 0) first, then odds ----
            t1 = [None] * W1C
            act_parts = {}
            for c in ACT_EVENS:          # ACT sign pairs emitted up front
                act_parts[c] = quant_act_signs(w1t[c], g1h, HC2)

            even_order = (0, 2, 4, 6, 8, 10)
            for c in even_order:
                t = t1p.tile([128, HC2], FP8, tag="t1")
                if c in act_parts:
                    a, b = act_parts[c]
                    nc.vector.tensor_sub(out=t, in0=a, in1=b)
                else:
                    quant_dve(w1t[c], t, g1h, g1n, HC2)
                t1[c] = t

            def t1_slice(hc, kd):
                c = kd * 2 + (hc * 128) // HC2
                off = (hc * 128) % HC2
                return t1[c][:, off:off + 128]

            hbt = {t: [None] * KH for t in range(NT)}

            def gelu_block(t, hcs):
                for hc in hcs:
                    ps = ps_open.pop(hc)
                    hbv = hbp.tile([128, TN], BF16, tag="hb")
                    nc.scalar.activation(hbv, ps, Act.Gelu,
                                         bias=b1sb[:, hc:hc + 1],
                                         scale=g1h[:, 0:1])
                    hbt[t][hc] = hbv

            ps_open = {}

            def fc1_chunk_major(t, hcs, chunk_order):
                """Open one psum per hc; each chunk contributes immediately."""
                tok = slice(t * TN, (t + 1) * TN)
                for hc in hcs:
                    ps_open[hc] = psp.tile([128, TN], FP32, tag="ps",
                                           name=f"hps_t{t}_hc{hc}")
                for j, c in enumerate(chunk_order):
                    kd = c // 2
                    for hc in hcs:
                        nc.tensor.matmul(ps_open[hc], t1_slice(hc, kd),
                                         xb[kd][:, tok],
                                         start=(j == 0), stop=(j == KD - 1))

            def fc1_hc_major(t, hcs):
                tok = slice(t * TN, (t + 1) * TN)
                for hc in hcs:
                    ps = psp.tile([128, TN], FP32, tag="ps")
                    for j in range(KD):
                        nc.tensor.matmul(ps, t1_slice(hc, j),
                                         xb[j][:, tok],
                                         start=(j == 0), stop=(j == KD - 1))
                    ps_open[hc] = ps
                gelu_block(t, hcs)

            # ---- phase A: chunk-major fc1 t0 hc0-7 over even chunks ----
            fc1_chunk_major(0, range(0, 8), even_order)
            gelu_block(0, range(0, 8))
            # ---- B: t0 hc8-11 ----
            fc1_hc_major(0, range(8, 12))

            # ---- odd w1 chunks (all DVE) ----
            for c in (1, 3, 5, 7, 9, 11):
                t = t1p.tile([128, HC2], FP8, tag="t1")
                quant_dve(w1t[c], t, g1h, g1n, HC2)
                t1[c] = t

            # ---- C: t1 hc0-11 ----
            fc1_hc_major(1, range(0, 12))
            # ---- D: chunk-major t0 hc12-19 over odd chunks ----
            fc1_chunk_major(0, range(12, 20), (1, 3, 5, 7, 9, 11))
            gelu_block(0, range(12, 20))
            # ---- E: t0 hc20-23 ----
            fc1_hc_major(0, range(20, 24))

            # ---- w2 reduces + gamma2 (DVE reaches here ~ after odds) ----
            acc2 = singles.tile([128, KH], FP32, tag="acc2")
            for bt in range(W2B):
                nc.vector.tensor_reduce(out=acc2[:, 4 * bt:4 * bt + 4],
                                        in_=w2t[bt], axis=AxX, op=Alu.add,
                                        apply_absolute_value=True)
            g2h, g2n = gamma_half(acc2, D * H, "g2")

            # ---- F: t1 hc12-23 ----
            fc1_hc_major(1, range(12, 24))

            # ---- w2 quant: DVE b0/b2/b4, ACT sign pairs b1/b3/b5 ----
            t2 = [None] * W2B

            def w2_quant_dve(bt):
                t = t2p.tile([128, 4, D], FP8, tag="t2")
                quant_dve(w2t[bt], t, g2h, g2n, 4 * D)
                t2[bt] = t

            def w2_quant_act(bt):
                a, b = quant_act_signs(w2t[bt], g2h, 4 * D)
                t = t2p.tile([128, 4, D], FP8, tag="t2")
                nc.vector.tensor_sub(out=t, in0=a, in1=b)
                t2[bt] = t

            w2_quant_act(1)
            w2_quant_dve(0)
            w2_quant_act(3)
            w2_quant_dve(2)
            w2_quant_act(5)
            w2_quant_dve(4)

            # ---- G: fc1 t2 full ----
            fc1_hc_major(2, range(0, KH))

            def fc2(t):
                tok = slice(t * TN, (t + 1) * TN)
                for dc in range(KD):
                    ps2 = psp.tile([128, TN], FP32, tag="ps")
                    for j in range(KH):
                        lhsT = t2[j // 4][:, j % 4, dc * 128:(dc + 1) * 128]
                        nc.tensor.matmul(ps2, lhsT, hbt[t][j],
                                         start=(j == 0), stop=(j == KH - 1))
                    ysb = ysbp.tile([128, TN], BF16, tag="ysb")
                    nc.vector.tensor_scalar(
                        out=ysb, in0=ps2, scalar1=g2h[:, 0:1],
                        scalar2=b2sb[:, dc:dc + 1],
                        op0=Alu.mult, op1=Alu.add)
                    nc.gpsimd.dma_start(out=yt[dc * 128:(dc + 1) * 128, tok],
                                        in_=ysb)
                for kh in range(KH):
                    hbt[t][kh] = None

            # ---- H..L ----
            fc2(0)
            fc1_hc_major(3, range(0, KH))
            fc2(1)
            fc2(2)
            fc2(3)

    nc.compile()
    return nc


_NC = None


def _get_nc():
    global _NC
    if _NC is None:
        _NC = build()
    return _NC


def kernel(x, w1, b1, w2, b2, _trace=False, _trace_kwargs=None):
    from concourse.bass_utils import run_bass_kernel_spmd
    nc = _get_nc()
    x = np.asarray(x, dtype=np.float32)
    w1 = np.asarray(w1, dtype=np.float32)
    b1 = np.asarray(b1, dtype=np.float32)
    w2 = np.asarray(w2, dtype=np.float32)
    b2 = np.asarray(b2, dtype=np.float32)
    x2 = np.ascontiguousarray(x.reshape(TOK, D).T).astype(ml_dtypes.bfloat16)
    wt1 = np.ascontiguousarray(w1.T).astype(np.float16)        # [768, 3072]
    wt2r = np.ascontiguousarray(
        w2.T.reshape(KH, 128, D).transpose(1, 0, 2)).astype(np.float16)
    b1r = np.ascontiguousarray(b1.reshape(KH, 128).T)          # [128, 24]
    b2r = np.ascontiguousarray(b2.reshape(KD, 128).T)          # [128, 6]
    in_maps = []
    for c in range(N_CORES):
        in_maps.append({
            "xt": np.ascontiguousarray(x2[:, c * TOK_PER:(c + 1) * TOK_PER]),
            "wt1": wt1, "wt2r": wt2r, "b1r": b1r, "b2r": b2r,
        })
    out = run_bass_kernel_spmd(nc, in_maps, list(range(N_CORES)),
                               trace=_trace, **(_trace_kwargs or {}))
    res = out.results
    yt = np.concatenate([res[c]["yt"].astype(np.float32)
                         for c in range(N_CORES)], axis=1)
    y = np.ascontiguousarray(yt.T).reshape(B, S, D)
    if _trace:
        return y, out
    return y


# revision 5
# speedup vs baseline: 1.1607x; 1.1162x over previous
"""Trainium2 Bass kernel for nn_Mlp_13099650253522 (BitNet-ternary dense MLP).

  h = gelu(x @ ter_quant(w1).T + b1);  y = h @ ter_quant(w2).T + b2
  ter_quant(w) = clip(round(w / g), -1, 1) * g,  g = mean(|w|) + 1e-5

Strategy (8 NeuronCores, data-parallel over the 64*197=12608 tokens):
 - Host: transpose + downcast weights to fp16 (layout/dtype only; ternary
   threshold classification verified numerically: rel err ~1.2% < 2e-2),
   x to bf16, shard tokens 1576/core. y returned bf16, upcast on host.
 - Device (per core, identical program):
     * w1 streams in 12 fp16 chunks; |row| sums split DVE (tensor_reduce)
       / ACT (Abs + accum_out) so the reduce chain tracks the DMA instead
       of lagging it; fused gamma chain ending in reciprocal(g).
     * ternary quant in TWO tensor_scalar ops per chunk: w*(1/g) -> int16
       (the HW convert rounds to nearest-even, matching jnp.round), then
       clip to [-1,1] -> fp8. No slow tensor_tensor combine.
     * fc1 phase A is chunk-major: 8 PSUM accumulation groups stay open
       so the PE starts right after the first quantized chunk.
     * fc2: PE matmuls fp8 lhsT x bf16 h; DVE epilogue -> bf16 y out.
     * w2 loads once (fp16, host pre-swizzled [128, 24, 768]), reduced
       and quantized in fc1's slack window; no second pass.
 - PE floor is ~189us (bf16 moving operand); everything else is
   scheduled to keep the PE gapless.
"""
import sys

for _p in ("/root/.axon_site", "/root/.axon_site/_ro/trn_rl_repo",
           "/root/.axon_site/_ro/pypackages", "/opt/trn_rl_repo"):
    if _p not in sys.path:
        sys.path.append(_p)

import ml_dtypes
import numpy as np

from concourse import bacc
import concourse.mybir as mybir
from concourse import bass_isa
from concourse.tile import TileContext

FP32 = mybir.dt.float32
FP16 = mybir.dt.float16
BF16 = mybir.dt.bfloat16
FP8 = mybir.dt.float8e4
I16 = mybir.dt.int16
Act = mybir.ActivationFunctionType
Alu = mybir.AluOpType
AxX = mybir.AxisListType.X

N_CORES = 8
B, S, D, H = 64, 197, 768, 3072
TOK = B * S                 # 12608
TOK_PER = TOK // N_CORES    # 1576
NT = 4                      # token tiles per core
TN = TOK_PER // NT          # 394
KD = D // 128               # 6
KH = H // 24                # unused
KH = H // 128               # 24
EPS = 1e-5

W1C = 12                    # w1 chunks [128, 1536]
HC2 = H // 2
W2B = 6                     # w2 batches [128, 4, 768]


def build():
    nc = bacc.Bacc("TRN2", target_bir_lowering=False, debug=False)
    xt = nc.declare_dram_parameter("xt", [D, TOK_PER], BF16, isOutput=False)
    wt1 = nc.declare_dram_parameter("wt1", [D, H], FP16, isOutput=False)
    wt2r = nc.declare_dram_parameter("wt2r", [128, KH, D], FP16, isOutput=False)
    b1r = nc.declare_dram_parameter("b1r", [128, KH], FP32, isOutput=False)
    b2r = nc.declare_dram_parameter("b2r", [128, KD], FP32, isOutput=False)
    yt = nc.declare_dram_parameter("yt", [D, TOK_PER], BF16, isOutput=True)

    with TileContext(nc) as tc:
        with (
            tc.tile_pool(name="singles", bufs=1) as singles,
            tc.tile_pool(name="w1p", bufs=W1C) as w1p,       # fp16 w1 resident
            tc.tile_pool(name="t1p", bufs=W1C) as t1p,       # fp8 ternary w1
            tc.tile_pool(name="w2p", bufs=W2B) as w2p,       # fp16 w2 resident
            tc.tile_pool(name="t2p", bufs=W2B) as t2p,       # fp8 ternary w2
            tc.tile_pool(name="xb", bufs=KD) as xbp,         # x bf16 resident
            tc.tile_pool(name="hb", bufs=74) as hbp,         # gelu outputs
            tc.tile_pool(name="scrD", bufs=2) as scrD,       # int16 round scratch
            tc.tile_pool(name="scrA", bufs=2) as scrA,       # fp8 junk for ACT reduce
            tc.tile_pool(name="ysb", bufs=3) as ysbp,
            tc.tile_pool(name="ps", bufs=8, space="PSUM") as psp,
        ):
            # warm the gpsimd custom-op library while w1 streams in
            dmy = singles.tile([128, 1], FP32, tag="dmy")
            nc.gpsimd.memset(dmy, 0.0)
            dmy2 = singles.tile([128, 1], FP32, tag="dmy2")
            nc.gpsimd.partition_all_reduce(dmy2, dmy, channels=128,
                                           reduce_op=bass_isa.ReduceOp.add)

            # biases via the gpsimd DMA queue (idle at start)
            b1sb = singles.tile([128, KH], FP32, tag="b1sb")
            nc.gpsimd.dma_start(out=b1sb, in_=b1r[:, :])
            b2sb = singles.tile([128, KD], FP32, tag="b2sb")
            nc.gpsimd.dma_start(out=b2sb, in_=b2r[:, :])

            # ---- w1 DMA (12 fp16 chunks) + reduces split DVE/ACT ----
            w1t = []
            acc1 = singles.tile([128, W1C], FP32, tag="acc1")
            for c in range(W1C):
                kd, half = c // 2, c % 2
                wf = w1p.tile([128, HC2], FP16, tag="w1")
                nc.sync.dma_start(
                    out=wf, in_=wt1[kd * 128:(kd + 1) * 128,
                                    half * HC2:(half + 1) * HC2])
                w1t.append(wf)
                if c % 2 == 0:
                    nc.vector.tensor_reduce(out=acc1[:, c:c + 1], in_=wf,
                                            axis=AxX, op=Alu.add,
                                            apply_absolute_value=True)
                else:
                    junk = scrA.tile([128, HC2], FP8, tag="scrA")
                    nc.scalar.activation(junk, wf, Act.Abs,
                                         accum_out=acc1[:, c:c + 1])

            # ---- x DMA (6 bf16 chunks, same queue => after w1) ----
            xb = []
            for kd in range(KD):
                xbt = xbp.tile([128, TOK_PER], BF16, tag="xb")
                nc.sync.dma_start(out=xbt, in_=xt[kd * 128:(kd + 1) * 128, :])
                xb.append(xbt)

            # ---- w2 DMA (6 fp16 batches, after x) ----
            w2t = []
            for bt in range(W2B):
                wf = w2p.tile([128, 4, D], FP16, tag="w2")
                nc.sync.dma_start(out=wf, in_=wt2r[:, 4 * bt:4 * bt + 4, :])
                w2t.append(wf)

            def gamma_chain(acc_cols, total_elems, tag):
                """|w| partial sums -> (g, 1/g) broadcast [128,1] fp32."""
                rsum = singles.tile([128, 1], FP32, tag=tag + "_rs")
                nc.vector.tensor_reduce(out=rsum[:, 0:1], in_=acc_cols,
                                        axis=AxX, op=Alu.add)
                allr = singles.tile([128, 1], FP32, tag=tag + "_ar")
                nc.gpsimd.partition_all_reduce(allr, rsum, channels=128,
                                               reduce_op=bass_isa.ReduceOp.add)
                gf = singles.tile([128, 1], FP32, tag=tag + "_gf")
                nc.vector.tensor_scalar(
                    out=gf, in0=allr, scalar1=1.0 / total_elems,
                    scalar2=EPS, op0=Alu.mult, op1=Alu.add)
                gi = singles.tile([128, 1], FP32, tag=tag + "_gi")
                nc.vector.reciprocal(gi, gf)
                return gf, gi

            # ---- gamma1 ----
            g1f, g1i = gamma_chain(acc1, D * H, "g1")

            def quant(wf, t, gi, n):
                """t = clip(round(w/g), -1, 1) in fp8 via int16 round."""
                r = scrD.tile([128, n], I16, tag="scrD")
                nc.vector.tensor_scalar(out=r, in0=wf, scalar1=gi[:, 0:1],
                                        scalar2=None, op0=Alu.mult)
                nc.vector.tensor_scalar(out=t, in0=r, scalar1=-1.0,
                                        scalar2=1.0, op0=Alu.max, op1=Alu.min)

            # ---- w1 quant: evens (kd half 0) first, then odds ----
            t1 = [None] * W1C
            even_order = (0, 2, 4, 6, 8, 10)
            for c in even_order:
                t = t1p.tile([128, HC2], FP8, tag="t1")
                quant(w1t[c], t, g1i, HC2)
                t1[c] = t

            def t1_slice(hc, kd):
                c = kd * 2 + (hc * 128) // HC2
                off = (hc * 128) % HC2
                return t1[c][:, off:off + 128]

            hbt = {t: [None] * KH for t in range(NT)}
            ps_open = {}

            def gelu_block(t, hcs):
                for hc in hcs:
                    ps = ps_open.pop(hc)
                    hbv = hbp.tile([128, TN], BF16, tag="hb")
                    nc.scalar.activation(hbv, ps, Act.Gelu,
                                         bias=b1sb[:, hc:hc + 1],
                                         scale=g1f[:, 0:1])
                    hbt[t][hc] = hbv

            def fc1_chunk_major(t, hcs, chunk_order):
                """Open one psum per hc; each chunk contributes immediately."""
                tok = slice(t * TN, (t + 1) * TN)
                for hc in hcs:
                    ps_open[hc] = psp.tile([128, TN], FP32, tag="ps",
                                           name=f"hps_t{t}_hc{hc}")
                for j, c in enumerate(chunk_order):
                    kd = c // 2
                    for hc in hcs:
                        nc.tensor.matmul(ps_open[hc], t1_slice(hc, kd),
                                         xb[kd][:, tok],
                                         start=(j == 0), stop=(j == KD - 1))

            def fc1_hc_major(t, hcs):
                tok = slice(t * TN, (t + 1) * TN)
                for hc in hcs:
                    ps = psp.tile([128, TN], FP32, tag="ps")
                    for j in range(KD):
                        nc.tensor.matmul(ps, t1_slice(hc, j),
                                         xb[j][:, tok],
                                         start=(j == 0), stop=(j == KD - 1))
                    ps_open[hc] = ps
                gelu_block(t, hcs)

            # ---- phase A: chunk-major fc1 t0 hc0-7 over even chunks ----
            fc1_chunk_major(0, range(0, 8), even_order)
            gelu_block(0, range(0, 8))
            # ---- B: t0 hc8-11 ----
            fc1_hc_major(0, range(8, 12))

            # ---- odd w1 chunks ----
            for c in (1, 3, 5, 7, 9, 11):
                t = t1p.tile([128, HC2], FP8, tag="t1")
                quant(w1t[c], t, g1i, HC2)
                t1[c] = t

            # ---- C: t1 hc0-11 ----
            fc1_hc_major(1, range(0, 12))
            # ---- D/E: t0 hc12-23 ----
            fc1_hc_major(0, range(12, 24))

            # ---- w2 reduces + gamma2 (DVE reaches here after odd quant) ----
            acc2 = singles.tile([128, KH], FP32, tag="acc2")
            for bt in range(W2B):
                nc.vector.tensor_reduce(out=acc2[:, 4 * bt:4 * bt + 4],
                                        in_=w2t[bt], axis=AxX, op=Alu.add,
                                        apply_absolute_value=True)
            g2f, g2i = gamma_chain(acc2, D * H, "g2")

            # ---- F: t1 hc12-23 ----
            fc1_hc_major(1, range(12, 24))

            # ---- w2 quant (all DVE, int16 round) ----
            t2 = [None] * W2B
            for bt in range(W2B):
                t = t2p.tile([128, 4, D], FP8, tag="t2")
                quant(w2t[bt], t, g2i, 4 * D)
                t2[bt] = t

            # ---- G: fc1 t2 full ----
            fc1_hc_major(2, range(0, KH))

            def fc2(t):
                tok = slice(t * TN, (t + 1) * TN)
                for dc in range(KD):
                    ps2 = psp.tile([128, TN], FP32, tag="ps")
                    for j in range(KH):
                        lhsT = t2[j // 4][:, j % 4, dc * 128:(dc + 1) * 128]
                        nc.tensor.matmul(ps2, lhsT, hbt[t][j],
                                         start=(j == 0), stop=(j == KH - 1))
                    ysb = ysbp.tile([128, TN], BF16, tag="ysb")
                    nc.vector.tensor_scalar(
                        out=ysb, in0=ps2, scalar1=g2f[:, 0:1],
                        scalar2=b2sb[:, dc:dc + 1],
                        op0=Alu.mult, op1=Alu.add)
                    nc.gpsimd.dma_start(out=yt[dc * 128:(dc + 1) * 128, tok],
                                        in_=ysb)
                for kh in range(KH):
                    hbt[t][kh] = None

            # ---- H..L ----
            fc2(0)
            fc1_hc_major(3, range(0, KH))
            fc2(1)
            fc2(2)
            fc2(3)

    nc.compile()
    return nc


_NC = None


def _get_nc():
    global _NC
    if _NC is None:
        _NC = build()
    return _NC


def kernel(x, w1, b1, w2, b2, _trace=False, _trace_kwargs=None):
    from concourse.bass_utils import run_bass_kernel_spmd
    nc = _get_nc()
    x = np.asarray(x, dtype=np.float32)
    w1 = np.asarray(w1, dtype=np.float32)
    b1 = np.asarray(b1, dtype=np.float32)
    w2 = np.asarray(w2, dtype=np.float32)
    b2 = np.asarray(b2, dtype=np.float32)
    x2 = np.ascontiguousarray(x.reshape(TOK, D).T).astype(ml_dtypes.bfloat16)
    wt1 = np.ascontiguousarray(w1.T).astype(np.float16)        # [768, 3072]
    wt2r = np.ascontiguousarray(
        w2.T.reshape(KH, 128, D).transpose(1, 0, 2)).astype(np.float16)
    b1r = np.ascontiguousarray(b1.reshape(KH, 128).T)          # [128, 24]
    b2r = np.ascontiguousarray(b2.reshape(KD, 128).T)          # [128, 6]
    in_maps = []
    for c in range(N_CORES):
        in_maps.append({
            "xt": np.ascontiguousarray(x2[:, c * TOK_PER:(c + 1) * TOK_PER]),
            "wt1": wt1, "wt2r": wt2r, "b1r": b1r, "b2r": b2r,
        })
    out = run_bass_kernel_spmd(nc, in_maps, list(range(N_CORES)),
                               trace=_trace, **(_trace_kwargs or {}))
    res = out.results
    yt = np.concatenate([res[c]["yt"].astype(np.float32)
                         for c in range(N_CORES)], axis=1)
    y = np.ascontiguousarray(yt.T).reshape(B, S, D)
    if _trace:
        return y, out
    return y


# revision 10
# speedup vs baseline: 1.1649x; 1.0036x over previous
"""Trainium2 Bass kernel for nn_Mlp_13099650253522 (BitNet-ternary dense MLP).

  h = gelu(x @ ter_quant(w1).T + b1);  y = h @ ter_quant(w2).T + b2
  ter_quant(w) = clip(round(w / g), -1, 1) * g,  g = mean(|w|) + 1e-5

Strategy (8 NeuronCores, data-parallel over the 64*197=12608 tokens):
 - Host: transpose + downcast weights to fp16 (layout/dtype only; ternary
   threshold classification verified numerically: rel err ~1.2% < 2e-2),
   x to bf16, shard tokens 1576/core. y returned bf16, upcast on host.
 - Device (per core, identical program):
     * w1 streams in 12 fp16 chunks; |row| sums split DVE (tensor_reduce)
       / ACT (Abs + accum_out) so the reduce chain tracks the DMA instead
       of lagging it; fused gamma chain ending in reciprocal(g).
     * ternary quant in TWO tensor_scalar ops per chunk: w*(1/g) -> int16
       (the HW convert rounds to nearest-even, matching jnp.round), then
       clip to [-1,1] -> fp8. No slow tensor_tensor combine.
     * fc1 phase A is chunk-major: 8 PSUM accumulation groups stay open
       so the PE starts right after the first quantized chunk.
     * fc2: PE matmuls fp8 lhsT x bf16 h; DVE epilogue -> bf16 y out.
     * w2 loads once (fp16, host pre-swizzled [128, 24, 768]), reduced
       and quantized in fc1's slack window; no second pass.
 - PE floor is ~189us (bf16 moving operand); everything else is
   scheduled to keep the PE gapless.
"""
import sys

for _p in ("/root/.axon_site", "/root/.axon_site/_ro/trn_rl_repo",
           "/root/.axon_site/_ro/pypackages", "/opt/trn_rl_repo"):
    if _p not in sys.path:
        sys.path.append(_p)

import ml_dtypes
import numpy as np

from concourse import bacc
import concourse.mybir as mybir
from concourse import bass_isa
from concourse.tile import TileContext

FP32 = mybir.dt.float32
FP16 = mybir.dt.float16
BF16 = mybir.dt.bfloat16
FP8 = mybir.dt.float8e4
I16 = mybir.dt.int16
Act = mybir.ActivationFunctionType
Alu = mybir.AluOpType
AxX = mybir.AxisListType.X

N_CORES = 8
B, S, D, H = 64, 197, 768, 3072
TOK = B * S                 # 12608
TOK_PER = TOK // N_CORES    # 1576
NT = 4                      # token tiles per core
TN = TOK_PER // NT          # 394
KD = D // 128               # 6
KH = H // 24                # unused
KH = H // 128               # 24
EPS = 1e-5

W1C = 12                    # w1 chunks [128, 1536]
HC2 = H // 2
W2B = 6                     # w2 batches [128, 4, 768]
WARM_MM = 104               # dummy matmuls to hold the PE clock at full speed
WARM_N = 512                # columns per warm matmul


def build():
    nc = bacc.Bacc("TRN2", target_bir_lowering=False, debug=False)
    xt = nc.declare_dram_parameter("xt", [D, TOK_PER], BF16, isOutput=False)
    wt1 = nc.declare_dram_parameter("wt1", [D, H], FP16, isOutput=False)
    wt2r = nc.declare_dram_parameter("wt2r", [128, KH, D], FP16, isOutput=False)
    b1r = nc.declare_dram_parameter("b1r", [128, KH], FP32, isOutput=False)
    b2r = nc.declare_dram_parameter("b2r", [128, KD], FP32, isOutput=False)
    yt = nc.declare_dram_parameter("yt", [D, TOK_PER], BF16, isOutput=True)

    with TileContext(nc) as tc:
        with (
            tc.tile_pool(name="singles", bufs=1) as singles,
            tc.tile_pool(name="w1p", bufs=W1C) as w1p,       # fp16 w1 resident
            tc.tile_pool(name="t1p", bufs=W1C) as t1p,       # fp8 ternary w1
            tc.tile_pool(name="w2p", bufs=W2B) as w2p,       # fp16 w2 resident
            tc.tile_pool(name="t2p", bufs=W2B) as t2p,       # fp8 ternary w2
            tc.tile_pool(name="xb", bufs=KD) as xbp,         # x bf16 resident
            tc.tile_pool(name="hb", bufs=74) as hbp,         # gelu outputs
            tc.tile_pool(name="scrD", bufs=2) as scrD,       # int16 round scratch
            tc.tile_pool(name="scrA", bufs=2) as scrA,       # fp8 junk for ACT reduce
            tc.tile_pool(name="ysb", bufs=3) as ysbp,
            tc.tile_pool(name="ps", bufs=8, space="PSUM") as psp,
        ):
            # warm the gpsimd custom-op library while w1 streams in
            dmy = singles.tile([128, 1], FP32, tag="dmy")
            nc.gpsimd.memset(dmy, 0.0)
            dmy2 = singles.tile([128, 1], FP32, tag="dmy2")
            nc.gpsimd.partition_all_reduce(dmy2, dmy, channels=128,
                                           reduce_op=bass_isa.ReduceOp.add)

            # PE pre-warm: keep the tensor engine clocked up during the w1
            # DMA phase so real matmuls start at full DVFS speed.
            wlhs = singles.tile([128, 128], FP8, tag="wlhs")
            nc.vector.memset(wlhs, 0.0)
            wrhs = singles.tile([128, WARM_N], BF16, tag="wrhs")
            nc.vector.memset(wrhs, 0.0)
            wps = psp.tile([128, WARM_N], FP32, tag="ps")
            for _ in range(WARM_MM):
                nc.tensor.matmul(wps, wlhs, wrhs, start=True, stop=True)

            # biases via the gpsimd DMA queue (idle at start)
            b1sb = singles.tile([128, KH], FP32, tag="b1sb")
            nc.gpsimd.dma_start(out=b1sb, in_=b1r[:, :])
            b2sb = singles.tile([128, KD], FP32, tag="b2sb")
            nc.gpsimd.dma_start(out=b2sb, in_=b2r[:, :])

            # ---- w1 DMA (12 fp16 chunks) + reduces split DVE/ACT ----
            w1t = []
            acc1 = singles.tile([128, W1C], FP32, tag="acc1")
            for c in range(W1C):
                kd, half = c // 2, c % 2
                wf = w1p.tile([128, HC2], FP16, tag="w1")
                nc.sync.dma_start(
                    out=wf, in_=wt1[kd * 128:(kd + 1) * 128,
                                    half * HC2:(half + 1) * HC2])
                w1t.append(wf)
                if c % 2 == 0 or c == 11:
                    nc.vector.tensor_reduce(out=acc1[:, c:c + 1], in_=wf,
                                            axis=AxX, op=Alu.add,
                                            apply_absolute_value=True)
                else:
                    junk = scrA.tile([128, HC2], FP8, tag="scrA")
                    nc.scalar.activation(junk, wf, Act.Abs,
                                         accum_out=acc1[:, c:c + 1])

            # ---- x DMA (6 bf16 chunks, same queue => after w1) ----
            xb = []
            for kd in range(KD):
                xbt = xbp.tile([128, TOK_PER], BF16, tag="xb")
                nc.sync.dma_start(out=xbt, in_=xt[kd * 128:(kd + 1) * 128, :])
                xb.append(xbt)

            # ---- w2 DMA (6 fp16 batches, after x) ----
            w2t = []
            for bt in range(W2B):
                wf = w2p.tile([128, 4, D], FP16, tag="w2")
                nc.sync.dma_start(out=wf, in_=wt2r[:, 4 * bt:4 * bt + 4, :])
                w2t.append(wf)

            def gamma_chain(acc_cols, total_elems, tag):
                """|w| partial sums -> (g, 1/g) broadcast [128,1] fp32."""
                rsum = singles.tile([128, 1], FP32, tag=tag + "_rs")
                nc.vector.tensor_reduce(out=rsum[:, 0:1], in_=acc_cols,
                                        axis=AxX, op=Alu.add)
                allr = singles.tile([128, 1], FP32, tag=tag + "_ar")
                nc.gpsimd.partition_all_reduce(allr, rsum, channels=128,
                                               reduce_op=bass_isa.ReduceOp.add)
                gf = singles.tile([128, 1], FP32, tag=tag + "_gf")
                nc.vector.tensor_scalar(
                    out=gf, in0=allr, scalar1=1.0 / total_elems,
                    scalar2=EPS, op0=Alu.mult, op1=Alu.add)
                gi = singles.tile([128, 1], FP32, tag=tag + "_gi")
                nc.vector.reciprocal(gi, gf)
                return gf, gi

            # ---- gamma1 ----
            g1f, g1i = gamma_chain(acc1, D * H, "g1")

            def quant(wf, t, gi, n):
                """t = clip(round(w/g), -1, 1) in fp8 via int16 round."""
                r = scrD.tile([128, n], I16, tag="scrD")
                nc.vector.tensor_scalar(out=r, in0=wf, scalar1=gi[:, 0:1],
                                        scalar2=None, op0=Alu.mult)
                nc.vector.tensor_scalar(out=t, in0=r, scalar1=-1.0,
                                        scalar2=1.0, op0=Alu.max, op1=Alu.min)

            # ---- w1 quant: evens (kd half 0) first, then odds ----
            t1 = [None] * W1C
            even_order = (0, 2, 4, 6, 8, 10)
            for c in even_order:
                t = t1p.tile([128, HC2], FP8, tag="t1")
                quant(w1t[c], t, g1i, HC2)
                t1[c] = t

            def t1_slice(hc, kd):
                c = kd * 2 + (hc * 128) // HC2
                off = (hc * 128) % HC2
                return t1[c][:, off:off + 128]

            hbt = {t: [None] * KH for t in range(NT)}
            ps_open = {}

            def gelu_block(t, hcs):
                for hc in hcs:
                    ps = ps_open.pop(hc)
                    hbv = hbp.tile([128, TN], BF16, tag="hb")
                    nc.scalar.activation(hbv, ps, Act.Gelu,
                                         bias=b1sb[:, hc:hc + 1],
                                         scale=g1f[:, 0:1])
                    hbt[t][hc] = hbv

            def fc1_chunk_major(t, hcs, chunk_order):
                """Open one psum per hc; each chunk contributes immediately."""
                tok = slice(t * TN, (t + 1) * TN)
                for hc in hcs:
                    ps_open[hc] = psp.tile([128, TN], FP32, tag="ps",
                                           name=f"hps_t{t}_hc{hc}")
                for j, c in enumerate(chunk_order):
                    kd = c // 2
                    for hc in hcs:
                        nc.tensor.matmul(ps_open[hc], t1_slice(hc, kd),
                                         xb[kd][:, tok],
                                         start=(j == 0), stop=(j == KD - 1))

            def fc1_hc_major(t, hcs):
                tok = slice(t * TN, (t + 1) * TN)
                for hc in hcs:
                    ps = psp.tile([128, TN], FP32, tag="ps")
                    for j in range(KD):
                        nc.tensor.matmul(ps, t1_slice(hc, j),
                                         xb[j][:, tok],
                                         start=(j == 0), stop=(j == KD - 1))
                    ps_open[hc] = ps
                gelu_block(t, hcs)

            # ---- phase A: chunk-major fc1 t0 hc0-7 over even chunks ----
            fc1_chunk_major(0, range(0, 8), even_order)
            gelu_block(0, range(0, 8))
            # ---- B: t0 hc8-11 ----
            fc1_hc_major(0, range(8, 12))

            # ---- odd w1 chunks ----
            for c in (1, 3, 5, 7, 9, 11):
                t = t1p.tile([128, HC2], FP8, tag="t1")
                quant(w1t[c], t, g1i, HC2)
                t1[c] = t

            # ---- C: t1 hc0-11 ----
            fc1_hc_major(1, range(0, 12))
            # ---- D/E: t0 hc12-23 ----
            fc1_hc_major(0, range(12, 24))

            # ---- w2 reduces + gamma2 (DVE reaches here after odd quant) ----
            acc2 = singles.tile([128, KH], FP32, tag="acc2")
            for bt in range(W2B):
                nc.vector.tensor_reduce(out=acc2[:, 4 * bt:4 * bt + 4],
                                        in_=w2t[bt], axis=AxX, op=Alu.add,
                                        apply_absolute_value=True)
            g2f, g2i = gamma_chain(acc2, D * H, "g2")

            # ---- F: t1 hc12-23 ----
            fc1_hc_major(1, range(12, 24))

            # ---- w2 quant (all DVE, int16 round) ----
            t2 = [None] * W2B
            for bt in range(W2B):
                t = t2p.tile([128, 4, D], FP8, tag="t2")
                quant(w2t[bt], t, g2i, 4 * D)
                t2[bt] = t

            # ---- G: fc1 t2 full ----
            fc1_hc_major(2, range(0, KH))

            def fc2(t):
                tok = slice(t * TN, (t + 1) * TN)
                for dc in range(KD):
                    ps2 = psp.tile([128, TN], FP32, tag="ps")
                    for j in range(KH):
                        lhsT = t2[j // 4][:, j % 4, dc * 128:(dc + 1) * 128]
                        nc.tensor.matmul(ps2, lhsT, hbt[t][j],
                                         start=(j == 0), stop=(j == KH - 1))
                    ysb = ysbp.tile([128, TN], BF16, tag="ysb")
                    nc.vector.tensor_scalar(
                        out=ysb, in0=ps2, scalar1=g2f[:, 0:1],
                        scalar2=b2sb[:, dc:dc + 1],
                        op0=Alu.mult, op1=Alu.add)
                    nc.gpsimd.dma_start(out=yt[dc * 128:(dc + 1) * 128, tok],
                                        in_=ysb)
                for kh in range(KH):
                    hbt[t][kh] = None

            # ---- H..L ----
            fc2(0)
            fc1_hc_major(3, range(0, KH))
            fc2(1)
            fc2(2)
            fc2(3)

    nc.compile()
    return nc


_NC = None


def _get_nc():
    global _NC
    if _NC is None:
        _NC = build()
    return _NC


def kernel(x, w1, b1, w2, b2, _trace=False, _trace_kwargs=None):
    from concourse.bass_utils import run_bass_kernel_spmd
    nc = _get_nc()
    x = np.asarray(x, dtype=np.float32)
    w1 = np.asarray(w1, dtype=np.float32)
    b1 = np.asarray(b1, dtype=np.float32)
    w2 = np.asarray(w2, dtype=np.float32)
    b2 = np.asarray(b2, dtype=np.float32)
    x2 = np.ascontiguousarray(x.reshape(TOK, D).T).astype(ml_dtypes.bfloat16)
    wt1 = np.ascontiguousarray(w1.T).astype(np.float16)        # [768, 3072]
    wt2r = np.ascontiguousarray(
        w2.T.reshape(KH, 128, D).transpose(1, 0, 2)).astype(np.float16)
    b1r = np.ascontiguousarray(b1.reshape(KH, 128).T)          # [128, 24]
    b2r = np.ascontiguousarray(b2.reshape(KD, 128).T)          # [128, 6]
    in_maps = []
    for c in range(N_CORES):
        in_maps.append({
            "xt": np.ascontiguousarray(x2[:, c * TOK_PER:(c + 1) * TOK_PER]),
            "wt1": wt1, "wt2r": wt2r, "b1r": b1r, "b2r": b2r,
        })
    out = run_bass_kernel_spmd(nc, in_maps, list(range(N_CORES)),
                               trace=_trace, **(_trace_kwargs or {}))
    res = out.results
    yt = np.concatenate([res[c]["yt"].astype(np.float32)
                         for c in range(N_CORES)], axis=1)
    y = np.ascontiguousarray(yt.T).reshape(B, S, D)
    if _trace:
        return y, out
    return y


# revision 11
# speedup vs baseline: 1.1674x; 1.0021x over previous
"""Trainium2 Bass kernel for nn_Mlp_13099650253522 (BitNet-ternary dense MLP).

  h = gelu(x @ ter_quant(w1).T + b1);  y = h @ ter_quant(w2).T + b2
  ter_quant(w) = clip(round(w / g), -1, 1) * g,  g = mean(|w|) + 1e-5

Strategy (8 NeuronCores, data-parallel over the 64*197=12608 tokens):
 - Host: transpose + downcast weights to fp16 (layout/dtype only; ternary
   threshold classification verified numerically: rel err ~1.2% < 2e-2),
   x to bf16, shard tokens 1576/core. y returned bf16, upcast on host.
 - Device (per core, identical program):
     * w1 streams in 12 fp16 chunks; |row| sums split DVE (tensor_reduce)
       / ACT (Abs + accum_out) so the reduce chain tracks the DMA instead
       of lagging it; fused gamma chain ending in reciprocal(g).
     * ternary quant in TWO tensor_scalar ops per chunk: w*(1/g) -> int16
       (the HW convert rounds to nearest-even, matching jnp.round), then
       clip to [-1,1] -> fp8. No slow tensor_tensor combine.
     * fc1 phase A is chunk-major: 8 PSUM accumulation groups stay open
       so the PE starts right after the first quantized chunk.
     * fc2: PE matmuls fp8 lhsT x bf16 h; DVE epilogue -> bf16 y out.
     * w2 loads once (fp16, host pre-swizzled [128, 24, 768]), reduced
       and quantized in fc1's slack window; no second pass.
 - PE floor is ~189us (bf16 moving operand); everything else is
   scheduled to keep the PE gapless.
"""
import sys

for _p in ("/root/.axon_site", "/root/.axon_site/_ro/trn_rl_repo",
           "/root/.axon_site/_ro/pypackages", "/opt/trn_rl_repo"):
    if _p not in sys.path:
        sys.path.append(_p)

import ml_dtypes
import numpy as np

from concourse import bacc
import concourse.mybir as mybir
from concourse import bass_isa
from concourse.tile import TileContext

FP32 = mybir.dt.float32
FP16 = mybir.dt.float16
BF16 = mybir.dt.bfloat16
FP8 = mybir.dt.float8e4
I16 = mybir.dt.int16
Act = mybir.ActivationFunctionType
Alu = mybir.AluOpType
AxX = mybir.AxisListType.X

N_CORES = 8
B, S, D, H = 64, 197, 768, 3072
TOK = B * S                 # 12608
TOK_PER = TOK // N_CORES    # 1576
NT = 4                      # token tiles per core
TN = TOK_PER // NT          # 394
KD = D // 128               # 6
KH = H // 24                # unused
KH = H // 128               # 24
EPS = 1e-5

W1C = 12                    # w1 chunks [128, 1536]
HC2 = H // 2
W2B = 6                     # w2 batches [128, 4, 768]
WARM_MM = 95                # dummy matmuls to hold the PE clock at full speed
WARM_N = 512                # columns per warm matmul


def build():
    nc = bacc.Bacc("TRN2", target_bir_lowering=False, debug=False)
    xt = nc.declare_dram_parameter("xt", [D, TOK_PER], BF16, isOutput=False)
    wt1 = nc.declare_dram_parameter("wt1", [D, H], FP16, isOutput=False)
    wt2r = nc.declare_dram_parameter("wt2r", [128, KH, D], FP16, isOutput=False)
    b1r = nc.declare_dram_parameter("b1r", [128, KH], FP32, isOutput=False)
    b2r = nc.declare_dram_parameter("b2r", [128, KD], FP32, isOutput=False)
    yt = nc.declare_dram_parameter("yt", [D, TOK_PER], BF16, isOutput=True)

    with TileContext(nc) as tc:
        with (
            tc.tile_pool(name="singles", bufs=1) as singles,
            tc.tile_pool(name="w1p", bufs=W1C) as w1p,       # fp16 w1 resident
            tc.tile_pool(name="t1p", bufs=W1C) as t1p,       # fp8 ternary w1
            tc.tile_pool(name="w2p", bufs=W2B) as w2p,       # fp16 w2 resident
            tc.tile_pool(name="t2p", bufs=W2B) as t2p,       # fp8 ternary w2
            tc.tile_pool(name="xb", bufs=KD) as xbp,         # x bf16 resident
            tc.tile_pool(name="hb", bufs=74) as hbp,         # gelu outputs
            tc.tile_pool(name="scrD", bufs=2) as scrD,       # int16 round scratch
            tc.tile_pool(name="scrA", bufs=2) as scrA,       # fp8 junk for ACT reduce
            tc.tile_pool(name="ysb", bufs=3) as ysbp,
            tc.tile_pool(name="ps", bufs=8, space="PSUM") as psp,
        ):
            # warm the gpsimd custom-op library while w1 streams in
            dmy = singles.tile([128, 1], FP32, tag="dmy")
            nc.gpsimd.memset(dmy, 0.0)
            dmy2 = singles.tile([128, 1], FP32, tag="dmy2")
            nc.gpsimd.partition_all_reduce(dmy2, dmy, channels=128,
                                           reduce_op=bass_isa.ReduceOp.add)

            # PE pre-warm: keep the tensor engine clocked up during the w1
            # DMA phase so real matmuls start at full DVFS speed.
            wlhs = singles.tile([128, 128], FP8, tag="wlhs")
            nc.vector.memset(wlhs, 0.0)
            wrhs = singles.tile([128, WARM_N], BF16, tag="wrhs")
            nc.vector.memset(wrhs, 0.0)
            wps = psp.tile([128, WARM_N], FP32, tag="ps")
            for _ in range(WARM_MM):
                nc.tensor.matmul(wps, wlhs, wrhs, start=True, stop=True)

            # biases via the gpsimd DMA queue (idle at start)
            b1sb = singles.tile([128, KH], FP32, tag="b1sb")
            nc.gpsimd.dma_start(out=b1sb, in_=b1r[:, :])
            b2sb = singles.tile([128, KD], FP32, tag="b2sb")
            nc.gpsimd.dma_start(out=b2sb, in_=b2r[:, :])

            # ---- w1 DMA (12 fp16 chunks) + reduces split DVE/ACT ----
            w1t = []
            acc1 = singles.tile([128, W1C], FP32, tag="acc1")
            for c in range(W1C):
                kd, half = c // 2, c % 2
                wf = w1p.tile([128, HC2], FP16, tag="w1")
                nc.sync.dma_start(
                    out=wf, in_=wt1[kd * 128:(kd + 1) * 128,
                                    half * HC2:(half + 1) * HC2])
                w1t.append(wf)
                if c % 2 == 0 or c == 11:
                    nc.vector.tensor_reduce(out=acc1[:, c:c + 1], in_=wf,
                                            axis=AxX, op=Alu.add,
                                            apply_absolute_value=True)
                else:
                    junk = scrA.tile([128, HC2], FP8, tag="scrA")
                    nc.scalar.activation(junk, wf, Act.Abs,
                                         accum_out=acc1[:, c:c + 1])

            # ---- x DMA (6 bf16 chunks, same queue => after w1) ----
            xb = []
            for kd in range(KD):
                xbt = xbp.tile([128, TOK_PER], BF16, tag="xb")
                nc.sync.dma_start(out=xbt, in_=xt[kd * 128:(kd + 1) * 128, :])
                xb.append(xbt)

            # ---- w2 DMA (6 fp16 batches, after x) ----
            w2t = []
            for bt in range(W2B):
                wf = w2p.tile([128, 4, D], FP16, tag="w2")
                nc.sync.dma_start(out=wf, in_=wt2r[:, 4 * bt:4 * bt + 4, :])
                w2t.append(wf)

            def gamma_chain(acc_cols, total_elems, tag):
                """|w| partial sums -> (g, 1/g) broadcast [128,1] fp32."""
                rsum = singles.tile([128, 1], FP32, tag=tag + "_rs")
                nc.vector.tensor_reduce(out=rsum[:, 0:1], in_=acc_cols,
                                        axis=AxX, op=Alu.add)
                allr = singles.tile([128, 1], FP32, tag=tag + "_ar")
                nc.gpsimd.partition_all_reduce(allr, rsum, channels=128,
                                               reduce_op=bass_isa.ReduceOp.add)
                gf = singles.tile([128, 1], FP32, tag=tag + "_gf")
                nc.vector.tensor_scalar(
                    out=gf, in0=allr, scalar1=1.0 / total_elems,
                    scalar2=EPS, op0=Alu.mult, op1=Alu.add)
                gi = singles.tile([128, 1], FP32, tag=tag + "_gi")
                nc.vector.reciprocal(gi, gf)
                return gf, gi

            # ---- gamma1 ----
            g1f, g1i = gamma_chain(acc1, D * H, "g1")

            def quant(wf, t, gi, n):
                """t = clip(round(w/g), -1, 1) in fp8 via int16 round."""
                r = scrD.tile([128, n], I16, tag="scrD")
                nc.vector.tensor_scalar(out=r, in0=wf, scalar1=gi[:, 0:1],
                                        scalar2=None, op0=Alu.mult)
                nc.vector.tensor_scalar(out=t, in0=r, scalar1=-1.0,
                                        scalar2=1.0, op0=Alu.max, op1=Alu.min)

            # ---- w1 quant: evens (kd half 0) first, then odds ----
            t1 = [None] * W1C
            even_order = (0, 2, 4, 6, 8, 10)
            for c in even_order:
                t = t1p.tile([128, HC2], FP8, tag="t1")
                quant(w1t[c], t, g1i, HC2)
                t1[c] = t

            def t1_slice(hc, kd):
                c = kd * 2 + (hc * 128) // HC2
                off = (hc * 128) % HC2
                return t1[c][:, off:off + 128]

            hbt = {t: [None] * KH for t in range(NT)}
            ps_open = {}

            def gelu_block(t, hcs):
                for hc in hcs:
                    ps = ps_open.pop(hc)
                    hbv = hbp.tile([128, TN], BF16, tag="hb")
                    nc.scalar.activation(hbv, ps, Act.Gelu,
                                         bias=b1sb[:, hc:hc + 1],
                                         scale=g1f[:, 0:1])
                    hbt[t][hc] = hbv

            def fc1_chunk_major(t, hcs, chunk_order):
                """Open one psum per hc; each chunk contributes immediately."""
                tok = slice(t * TN, (t + 1) * TN)
                for hc in hcs:
                    ps_open[hc] = psp.tile([128, TN], FP32, tag="ps",
                                           name=f"hps_t{t}_hc{hc}")
                for j, c in enumerate(chunk_order):
                    kd = c // 2
                    for hc in hcs:
                        nc.tensor.matmul(ps_open[hc], t1_slice(hc, kd),
                                         xb[kd][:, tok],
                                         start=(j == 0), stop=(j == KD - 1))

            def fc1_hc_major(t, hcs):
                tok = slice(t * TN, (t + 1) * TN)
                for hc in hcs:
                    ps = psp.tile([128, TN], FP32, tag="ps")
                    for j in range(KD):
                        nc.tensor.matmul(ps, t1_slice(hc, j),
                                         xb[j][:, tok],
                                         start=(j == 0), stop=(j == KD - 1))
                    ps_open[hc] = ps
                gelu_block(t, hcs)

            # ---- phase A: chunk-major fc1 t0 hc0-7 over even chunks ----
            fc1_chunk_major(0, range(0, 8), even_order)
            gelu_block(0, range(0, 8))
            # ---- B: t0 hc8-11 ----
            fc1_hc_major(0, range(8, 12))

            # ---- odd w1 chunks ----
            for c in (1, 3, 5, 7, 9, 11):
                t = t1p.tile([128, HC2], FP8, tag="t1")
                quant(w1t[c], t, g1i, HC2)
                t1[c] = t

            # ---- C: t1 hc0-11 ----
            fc1_hc_major(1, range(0, 12))
            # ---- D/E: t0 hc12-23 ----
            fc1_hc_major(0, range(12, 24))

            # ---- w2 reduces + gamma2 (DVE reaches here after odd quant) ----
            acc2 = singles.tile([128, KH], FP32, tag="acc2")
            for bt in range(W2B):
                nc.vector.tensor_reduce(out=acc2[:, 4 * bt:4 * bt + 4],
                                        in_=w2t[bt], axis=AxX, op=Alu.add,
                                        apply_absolute_value=True)
            g2f, g2i = gamma_chain(acc2, D * H, "g2")

            # ---- F: t1 hc12-23 ----
            fc1_hc_major(1, range(12, 24))

            # ---- w2 quant (all DVE, int16 round) ----
            t2 = [None] * W2B
            for bt in range(W2B):
                t = t2p.tile([128, 4, D], FP8, tag="t2")
                quant(w2t[bt], t, g2i, 4 * D)
                t2[bt] = t

            # ---- G: fc1 t2 full ----
            fc1_hc_major(2, range(0, KH))

            def fc2(t):
                tok = slice(t * TN, (t + 1) * TN)
                for dc in range(KD):
                    ps2 = psp.tile([128, TN], FP32, tag="ps")
                    for j in range(KH):
                        lhsT = t2[j // 4][:, j % 4, dc * 128:(dc + 1) * 128]
                        nc.tensor.matmul(ps2, lhsT, hbt[t][j],
                                         start=(j == 0), stop=(j == KH - 1))
                    ysb = ysbp.tile([128, TN], BF16, tag="ysb")
                    nc.vector.tensor_scalar(
                        out=ysb, in0=ps2, scalar1=g2f[:, 0:1],
                        scalar2=b2sb[:, dc:dc + 1],
                        op0=Alu.mult, op1=Alu.add)
                    nc.gpsimd.dma_start(out=yt[dc * 128:(dc + 1) * 128, tok],
                                        in_=ysb)
                for kh in range(KH):
                    hbt[t][kh] = None

            # ---- H..L ----
            fc2(0)
            fc1_hc_major(3, range(0, KH))
            fc2(1)
            fc2(2)
            fc2(3)

    nc.compile()
    return nc


_NC = None


def _get_nc():
    global _NC
    if _NC is None:
        _NC = build()
    return _NC


def kernel(x, w1, b1, w2, b2, _trace=False, _trace_kwargs=None):
    from concourse.bass_utils import run_bass_kernel_spmd
    nc = _get_nc()
    x = np.asarray(x, dtype=np.float32)
    w1 = np.asarray(w1, dtype=np.float32)
    b1 = np.asarray(b1, dtype=np.float32)
    w2 = np.asarray(w2, dtype=np.float32)
    b2 = np.asarray(b2, dtype=np.float32)
    x2 = np.ascontiguousarray(x.reshape(TOK, D).T).astype(ml_dtypes.bfloat16)
    wt1 = np.ascontiguousarray(w1.T).astype(np.float16)        # [768, 3072]
    wt2r = np.ascontiguousarray(
        w2.T.reshape(KH, 128, D).transpose(1, 0, 2)).astype(np.float16)
    b1r = np.ascontiguousarray(b1.reshape(KH, 128).T)          # [128, 24]
    b2r = np.ascontiguousarray(b2.reshape(KD, 128).T)          # [128, 6]
    in_maps = []
    for c in range(N_CORES):
        in_maps.append({
            "xt": np.ascontiguousarray(x2[:, c * TOK_PER:(c + 1) * TOK_PER]),
            "wt1": wt1, "wt2r": wt2r, "b1r": b1r, "b2r": b2r,
        })
    out = run_bass_kernel_spmd(nc, in_maps, list(range(N_CORES)),
                               trace=_trace, **(_trace_kwargs or {}))
    res = out.results
    yt = np.concatenate([res[c]["yt"].astype(np.float32)
                         for c in range(N_CORES)], axis=1)
    y = np.ascontiguousarray(yt.T).reshape(B, S, D)
    if _trace:
        return y, out
    return y
